# revision 1
# baseline (speedup 1.0000x reference)
"""Trainium2 Bass kernel for nn_BoundingBoxDiscipline.

Computes PENALTY_WEIGHT * mean_B(area_penalty + center_offset) where the
penalties are derived from per-sample bounding boxes of thresholded masks:
    pred_mask = max_C(prediction_probs) > 0.3
    true_mask = max_C(expected_onehot)  > 0.5

The bounding box of a [H, W] mask only needs two tiny reductions:
    row_any[y] = any_{x,c} (v[y,x,c] > T)
    col_any[x] = any_{y,c} (v[y,x,c] > T)
so the device reduces 512 MiB of input down to 1 KiB of row/col summaries
per sample and the exact bbox/penalty math happens on the host (all
comparisons are exact: v > T <=> relu(v - T) > 0 in fp32, and sums of
non-negative values are > 0 iff any element is > 0).

Device plan, data-parallel over batch (2 samples per core, 8 cores):
  - DMA (the roofline: 64 MiB/core at ~342 GB/s measured): 16 tiles of
    [128 y-rows, 8192 (x*16+c)] fp32 per core.
  - ScalarE: relu(v - T) -> bf16 tile, with accum_out giving the per-row
    (free-axis) sum in one pass -> row_any.
  - VectorE: pairwise max folds channels 16 -> 4 (positivity-preserving),
    keeping the cold-clocked TensorE off the critical path.
  - TensorE: ones[128,1].T @ folded[:, :, c] accumulated over 4 y-tiles
    and 4 channel views into one PSUM [1, 512] -> col sums.
The last two tiles are split into 8 chunks each (chunk pool bufs=8) so the
trailing ScalarE work drains while the final bytes stream in instead of
stalling the chunk DMAs. Measured 201.7-202.9 us/iter steady-state vs a
196.2 us DMA-only floor (342 GB/s/core effective; 358 GB/s HBM spec).
"""

import time

import numpy as np

import concourse.bacc as bacc
import concourse.tile as tile
from concourse import mybir
from concourse.bass_utils import run_bass_kernel_spmd

N_CORES = 8
B, H, W, C = 16, 512, 512, 16
SAMPLES_PER_CORE = B // N_CORES          # 2
TILES_PER_SAMPLE = H // 128              # 4
FREE = W * C                             # 8192
THRESHOLDS = (0.3, 0.5)                  # (prediction_probs, expected_onehot)
PENALTY_WEIGHT = 0.05

f32 = mybir.dt.float32
bf16 = mybir.dt.bfloat16


def build_nc(repeat: int = 1, do_act: bool = True, do_mm: bool = True,
             c_fold: int = 4, dma_alt: bool = False):
    """Build the per-core Bass module. `repeat` wraps the body in a device
    loop; `do_act`/`do_mm` exist only for ablation timing experiments.
    `c_fold` is the channel count after DVE pairwise max-folding (16 = no
    fold); folding moves colsum work off the cold-clocked TensorE onto the
    otherwise-idle DVE. The graded path uses the defaults."""
    assert c_fold in (2, 4, 8, 16)
    nc = bacc.Bacc("TRN2", debug=False)

    n_tiles = 2 * SAMPLES_PER_CORE * TILES_PER_SAMPLE  # 16 (tensor, sample, ytile)
    n_st = 2 * SAMPLES_PER_CORE                        # 4 sample-tensors

    pred = nc.dram_tensor(
        "pred", [SAMPLES_PER_CORE * TILES_PER_SAMPLE, 128, FREE], f32,
        kind="ExternalInput").ap()
    exp = nc.dram_tensor(
        "exp", [SAMPLES_PER_CORE * TILES_PER_SAMPLE, 128, FREE], f32,
        kind="ExternalInput").ap()
    rows = nc.dram_tensor("rows", [128, n_tiles], f32, kind="ExternalOutput").ap()
    cols = nc.dram_tensor("cols", [n_st, W], f32, kind="ExternalOutput").ap()

    with tile.TileContext(nc) as tc:
        with (
            tc.tile_pool(name="singles", bufs=1) as singles,
            tc.tile_pool(name="loads", bufs=2) as loads,
            tc.tile_pool(name="relus", bufs=2) as relus,
            tc.tile_pool(name="rowsp", bufs=1) as rowsp,
            tc.tile_pool(name="chunkp", bufs=8) as chunkp,
            tc.tile_pool(name="colsb", bufs=2) as colsb,
            tc.tile_pool(name="psum", bufs=2, space="PSUM") as psum,
        ):
            ones = singles.tile([128, 1], bf16)
            nc.vector.memset(ones, 1.0)
            biases = []
            for thr in THRESHOLDS:
                bias_t = singles.tile([128, 1], f32, tag=f"bias{thr}")
                nc.vector.memset(bias_t, -thr)
                biases.append(bias_t)
            rows_sb = rowsp.tile([128, n_tiles], f32)

            def fold_c(rl_flat, n_x, tag):
                """DVE pairwise max over channel halves: 16 -> c_fold chans.
                Positivity-preserving, so col_any is unchanged."""
                cur, cur_c = rl_flat, C
                while cur_c > c_fold:
                    nxt = cur_c // 2
                    out_t = relus.tile([128, n_x * nxt], bf16, tag=f"{tag}{nxt}")
                    cur3 = cur.rearrange("p (x c) -> p x c", c=cur_c)
                    out3 = out_t.rearrange("p (x c) -> p x c", c=nxt)
                    nc.vector.tensor_tensor(
                        out=out3, in0=cur3[:, :, 0:nxt], in1=cur3[:, :, nxt:cur_c],
                        op=mybir.AluOpType.max)
                    cur, cur_c = out_t, nxt
                return cur.rearrange("p (x c) -> p x c", c=cur_c), cur_c

            def body(_iv=None):
                # The last two tiles are processed in 4 free-dim chunks each:
                # every trailing ACT op is then ~1.9 us < the 2.9 us chunk DMA
                # cadence, so ScalarE never backlogs the tail and the
                # post-final-DMA compute is ~2 us instead of ~13 us.
                N_CHUNKS = 8
                N_CHUNK_TILES = 2
                CHUNK = FREE // N_CHUNKS          # 1024 free elems = 64 x's
                XC = CHUNK // C                   # 128
                last_scratch = rowsp.tile([128, N_CHUNK_TILES * N_CHUNKS], f32)

                for tensor_idx, src in ((0, pred), (1, exp)):
                    bias_t = biases[tensor_idx]
                    for s in range(SAMPLES_PER_CORE):
                        st = tensor_idx * SAMPLES_PER_CORE + s
                        is_last_st = st == 2 * SAMPLES_PER_CORE - 1
                        psum_t = psum.tile([1, W], f32)
                        if is_last_st and N_CHUNK_TILES >= TILES_PER_SAMPLE and do_mm:
                            # all tiles chunked -> no N=512 start=True matmul
                            # to clear the bank; zero it and accumulate onto
                            # zeros (correct under any has_written semantics)
                            nc.vector.memset(psum_t, 0.0)
                        for t in range(TILES_PER_SAMPLE):
                            k = tensor_idx * 8 + s * 4 + t
                            if is_last_st and t >= TILES_PER_SAMPLE - N_CHUNK_TILES:
                                ct = t - (TILES_PER_SAMPLE - N_CHUNK_TILES)
                                scr = last_scratch[:, ct * N_CHUNKS:(ct + 1) * N_CHUNKS]
                                is_last_tile = t == TILES_PER_SAMPLE - 1
                                for ch in range(N_CHUNKS):
                                    ldc = chunkp.tile([128, CHUNK], f32, tag="ldc")
                                    nc.sync.dma_start(
                                        out=ldc,
                                        in_=src[s * 4 + t, :,
                                                ch * CHUNK:(ch + 1) * CHUNK])
                                    rlc = chunkp.tile([128, CHUNK], bf16, tag="rlc")
                                    if do_act:
                                        nc.scalar.activation(
                                            out=rlc, in_=ldc,
                                            func=mybir.ActivationFunctionType.Relu,
                                            bias=bias_t, scale=1.0,
                                            accum_out=scr[:, ch : ch + 1],
                                        )
                                    if do_mm:
                                        rlc3, n_c = fold_c(rlc, XC, "foldc")
                                        for ci in range(n_c):
                                            nc.tensor.matmul(
                                                psum_t[:, ch * XC:(ch + 1) * XC],
                                                ones, rlc3[:, :, ci],
                                                start=False,
                                                stop=(is_last_tile
                                                      and ch == N_CHUNKS - 1
                                                      and ci == n_c - 1),
                                                # accumulation onto the
                                                # memset-zeroed bank; the sim's
                                                # bank-granular group assert
                                                # can't express this
                                                skip_group_check=(
                                                    N_CHUNK_TILES
                                                    >= TILES_PER_SAMPLE),
                                            )
                                if do_act:
                                    nc.vector.reduce_max(
                                        rows_sb[:, k : k + 1], scr,
                                        axis=mybir.AxisListType.X)
                            else:
                                ld = loads.tile([128, FREE], f32)
                                dma_eng = (nc.scalar if (dma_alt and (k % 2)) else nc.sync)
                                dma_eng.dma_start(out=ld, in_=src[s * 4 + t])
                                rl = relus.tile([128, FREE], bf16)
                                if do_act:
                                    nc.scalar.activation(
                                        out=rl, in_=ld,
                                        func=mybir.ActivationFunctionType.Relu,
                                        bias=bias_t, scale=1.0,
                                        accum_out=rows_sb[:, k : k + 1],
                                    )
                                if do_mm:
                                    rl3, n_c = fold_c(rl, W, "fold")
                                    for ci in range(n_c):
                                        nc.tensor.matmul(
                                            psum_t, ones, rl3[:, :, ci],
                                            start=(t == 0 and ci == 0),
                                            stop=(not is_last_st
                                                  and t == TILES_PER_SAMPLE - 1
                                                  and ci == n_c - 1),
                                        )
                        if do_mm:
                            csb = colsb.tile([1, W], f32)
                            nc.vector.tensor_copy(csb, psum_t)
                            nc.sync.dma_start(out=cols[st : st + 1], in_=csb)
                if not do_mm:
                    csb = colsb.tile([4, W], f32)
                    nc.vector.memset(csb, 1.0)
                    nc.sync.dma_start(out=cols, in_=csb)
                if not do_act:
                    nc.vector.memset(rows_sb[:, :1], 1.0)
                nc.sync.dma_start(out=rows, in_=rows_sb)

            if repeat == 1:
                body()
            else:
                with tc.For_i(0, repeat, 1,
                              hint_engines=(mybir.EngineType.PE,)) as iv:
                    body(iv)

    nc.compile()
    return nc


def _shard_inputs(prediction_probs, expected_onehot):
    p = np.ascontiguousarray(np.asarray(prediction_probs), dtype=np.float32)
    e = np.ascontiguousarray(np.asarray(expected_onehot), dtype=np.float32)
    p = p.reshape(N_CORES, SAMPLES_PER_CORE * TILES_PER_SAMPLE, 128, FREE)
    e = e.reshape(N_CORES, SAMPLES_PER_CORE * TILES_PER_SAMPLE, 128, FREE)
    return [{"pred": p[c], "exp": e[c]} for c in range(N_CORES)]


def _bbox_from_any(row_any, col_any):
    ys = np.nonzero(row_any)[0]
    xs = np.nonzero(col_any)[0]
    if ys.size == 0:
        return 0, 0, 1, 1
    return int(ys[0]), int(xs[0]), int(ys[-1]), int(xs[-1])


def _combine(results):
    """Host epilogue: exact bbox/penalty math from row/col summaries."""
    f = np.float32
    penalties = []
    for core in range(N_CORES):
        rows = results[core]["rows"]  # [128, 16]
        cols = results[core]["cols"]  # [4, 512]
        for s in range(SAMPLES_PER_CORE):
            boxes = []
            for tensor_idx in range(2):
                k0 = tensor_idx * 8 + s * 4
                row_any = rows[:, k0 : k0 + 4].T.ravel() > 0  # y = t*128 + p
                col_any = cols[tensor_idx * SAMPLES_PER_CORE + s] > 0
                boxes.append(_bbox_from_any(row_any, col_any))
            (py1, px1, py2, px2), (ty1, tx1, ty2, tx2) = boxes
            pred_area = f((py2 - py1 + 1) * (px2 - px1 + 1))
            true_area = f((ty2 - ty1 + 1) * (tx2 - tx1 + 1))
            area_penalty = f(max(f(0.0), f(pred_area - true_area))) / f(true_area + f(1.0))
            pcy, pcx = f(py1 + py2) / f(2.0), f(px1 + px2) / f(2.0)
            tcy, tcx = f(ty1 + ty2) / f(2.0), f(tx1 + tx2) / f(2.0)
            center_offset = np.sqrt(np.square(f(pcy - tcy)) + np.square(f(pcx - tcx))) / f(20.0)
            penalties.append(f(area_penalty + center_offset))
    mean = np.mean(np.asarray(penalties, dtype=np.float32), dtype=np.float32)
    return np.asarray(np.float32(PENALTY_WEIGHT) * mean, dtype=np.float32)


_NC_CACHE = {}


def kernel(prediction_probs, expected_onehot):
    if "nc" not in _NC_CACHE:
        _NC_CACHE["nc"] = build_nc()
    nc = _NC_CACHE["nc"]
    in_maps = _shard_inputs(prediction_probs, expected_onehot)
    last_exc = None
    for attempt in range(3):  # the axon device occasionally flakes transiently
        try:
            res = run_bass_kernel_spmd(nc, in_maps, core_ids=list(range(N_CORES)))
            return _combine(res.results)
        except Exception as e:  # noqa: BLE001
            last_exc = e
            try:
                # an NRT_EXEC_UNIT_UNRECOVERABLE poisons the PJRT mesh for
                # the whole process; dropping the backend forces a reconnect
                import jax.extend.backend

                jax.extend.backend.clear_backends()
            except Exception:  # noqa: BLE001
                pass
            time.sleep(5.0)
    raise last_exc



# revision 4
# speedup vs baseline: 19.1670x; 19.1670x over previous
"""Trainium2 Bass kernel for nn_BoundingBoxDiscipline.

Computes PENALTY_WEIGHT * mean_B(area_penalty + center_offset) where the
penalties are derived from per-sample bounding boxes of thresholded masks:
    pred_mask = max_C(prediction_probs) > 0.3
    true_mask = max_C(expected_onehot)  > 0.5

The result depends ONLY on the bbox corners: the first/last nonempty row
and whether columns 0 / 511 are nonempty. That admits an exact adaptive
two-phase algorithm:

Phase 1 (strip kernel, the fast path): each core reads only the 4 edge
rows {0, 1, 510, 511} of its 4 sample-tensors (512 KiB/core instead of
64 MiB/core) as one contiguous DMA, applies relu(v - T) on ScalarE with
accum_out giving per-row-chunk sums, and reduces the 16-channel blocks of
columns 0 and 511 on VectorE. On the host, for each sample-tensor:
  - row 0 (or 1) nonempty   -> y_min proven exactly;
  - row 511 (or 510)        -> y_max proven exactly;
  - a true pixel at x=0 in any strip row   -> x_min = 0 proven;
  - a true pixel at x=511 in any strip row -> x_max = 511 proven.
All comparisons are exact (v > T <=> relu(v - T) > 0 in fp32/bf16, and
sums/maxes of non-negative values are > 0 iff some element is > 0).

Phase 2 (full kernel, fallback): if ANY corner fails to resolve from the
strips (possible only for nearly-empty/adversarial masks, probability
~1e-9 per sample for random dense inputs), rerun with the full-read
kernel, which computes complete row/col summaries from all 512 MiB and is
exact for every input. The combined algorithm is therefore exact for all
inputs while reading 128x less data on typical ones.

Full-kernel device plan, data-parallel over batch (2 samples/core):
  - DMA (the roofline: 64 MiB/core at ~342 GB/s measured): 16 tiles of
    [128 y-rows, 8192 (x*16+c)] fp32 per core.
  - ScalarE: relu(v - T) -> bf16 tile, with accum_out giving the per-row
    (free-axis) sum in one pass -> row_any.
  - VectorE: pairwise max folds channels 16 -> 4 (positivity-preserving),
    keeping the cold-clocked TensorE off the critical path.
  - TensorE: ones[128,1].T @ folded[:, :, c] accumulated over 4 y-tiles
    and 4 channel views into one PSUM [1, 512] -> col sums.
Measured 201.7-202.9 us/iter steady-state vs a 196.2 us DMA-only floor.
"""

import time

import numpy as np

import concourse.bacc as bacc
import concourse.tile as tile
from concourse import mybir
from concourse.bass_utils import run_bass_kernel_spmd

N_CORES = 8
B, H, W, C = 16, 512, 512, 16
SAMPLES_PER_CORE = B // N_CORES          # 2
TILES_PER_SAMPLE = H // 128              # 4
FREE = W * C                             # 8192
THRESHOLDS = (0.3, 0.5)                  # (prediction_probs, expected_onehot)
PENALTY_WEIGHT = 0.05

f32 = mybir.dt.float32
bf16 = mybir.dt.bfloat16


def build_nc(repeat: int = 1, do_act: bool = True, do_mm: bool = True,
             c_fold: int = 4, dma_alt: bool = False):
    """Build the per-core Bass module. `repeat` wraps the body in a device
    loop; `do_act`/`do_mm` exist only for ablation timing experiments.
    `c_fold` is the channel count after DVE pairwise max-folding (16 = no
    fold); folding moves colsum work off the cold-clocked TensorE onto the
    otherwise-idle DVE. The graded path uses the defaults."""
    assert c_fold in (2, 4, 8, 16)
    nc = bacc.Bacc("TRN2", debug=False)

    n_tiles = 2 * SAMPLES_PER_CORE * TILES_PER_SAMPLE  # 16 (tensor, sample, ytile)
    n_st = 2 * SAMPLES_PER_CORE                        # 4 sample-tensors

    pred = nc.dram_tensor(
        "pred", [SAMPLES_PER_CORE * TILES_PER_SAMPLE, 128, FREE], f32,
        kind="ExternalInput").ap()
    exp = nc.dram_tensor(
        "exp", [SAMPLES_PER_CORE * TILES_PER_SAMPLE, 128, FREE], f32,
        kind="ExternalInput").ap()
    rows = nc.dram_tensor("rows", [128, n_tiles], f32, kind="ExternalOutput").ap()
    cols = nc.dram_tensor("cols", [n_st, W], f32, kind="ExternalOutput").ap()

    with tile.TileContext(nc) as tc:
        with (
            tc.tile_pool(name="singles", bufs=1) as singles,
            tc.tile_pool(name="loads", bufs=2) as loads,
            tc.tile_pool(name="relus", bufs=2) as relus,
            tc.tile_pool(name="rowsp", bufs=1) as rowsp,
            tc.tile_pool(name="chunkp", bufs=8) as chunkp,
            tc.tile_pool(name="colsb", bufs=2) as colsb,
            tc.tile_pool(name="psum", bufs=2, space="PSUM") as psum,
        ):
            ones = singles.tile([128, 1], bf16)
            nc.vector.memset(ones, 1.0)
            biases = []
            for thr in THRESHOLDS:
                bias_t = singles.tile([128, 1], f32, tag=f"bias{thr}")
                nc.vector.memset(bias_t, -thr)
                biases.append(bias_t)
            rows_sb = rowsp.tile([128, n_tiles], f32)

            def fold_c(rl_flat, n_x, tag):
                """DVE pairwise max over channel halves: 16 -> c_fold chans.
                Positivity-preserving, so col_any is unchanged."""
                cur, cur_c = rl_flat, C
                while cur_c > c_fold:
                    nxt = cur_c // 2
                    out_t = relus.tile([128, n_x * nxt], bf16, tag=f"{tag}{nxt}")
                    cur3 = cur.rearrange("p (x c) -> p x c", c=cur_c)
                    out3 = out_t.rearrange("p (x c) -> p x c", c=nxt)
                    nc.vector.tensor_tensor(
                        out=out3, in0=cur3[:, :, 0:nxt], in1=cur3[:, :, nxt:cur_c],
                        op=mybir.AluOpType.max)
                    cur, cur_c = out_t, nxt
                return cur.rearrange("p (x c) -> p x c", c=cur_c), cur_c

            def body(_iv=None):
                # The last two tiles are processed in 4 free-dim chunks each:
                # every trailing ACT op is then ~1.9 us < the 2.9 us chunk DMA
                # cadence, so ScalarE never backlogs the tail and the
                # post-final-DMA compute is ~2 us instead of ~13 us.
                N_CHUNKS = 8
                N_CHUNK_TILES = 2
                CHUNK = FREE // N_CHUNKS          # 1024 free elems = 64 x's
                XC = CHUNK // C                   # 128
                last_scratch = rowsp.tile([128, N_CHUNK_TILES * N_CHUNKS], f32)

                for tensor_idx, src in ((0, pred), (1, exp)):
                    bias_t = biases[tensor_idx]
                    for s in range(SAMPLES_PER_CORE):
                        st = tensor_idx * SAMPLES_PER_CORE + s
                        is_last_st = st == 2 * SAMPLES_PER_CORE - 1
                        psum_t = psum.tile([1, W], f32)
                        if is_last_st and N_CHUNK_TILES >= TILES_PER_SAMPLE and do_mm:
                            # all tiles chunked -> no N=512 start=True matmul
                            # to clear the bank; zero it and accumulate onto
                            # zeros (correct under any has_written semantics)
                            nc.vector.memset(psum_t, 0.0)
                        for t in range(TILES_PER_SAMPLE):
                            k = tensor_idx * 8 + s * 4 + t
                            if is_last_st and t >= TILES_PER_SAMPLE - N_CHUNK_TILES:
                                ct = t - (TILES_PER_SAMPLE - N_CHUNK_TILES)
                                scr = last_scratch[:, ct * N_CHUNKS:(ct + 1) * N_CHUNKS]
                                is_last_tile = t == TILES_PER_SAMPLE - 1
                                for ch in range(N_CHUNKS):
                                    ldc = chunkp.tile([128, CHUNK], f32, tag="ldc")
                                    nc.sync.dma_start(
                                        out=ldc,
                                        in_=src[s * 4 + t, :,
                                                ch * CHUNK:(ch + 1) * CHUNK])
                                    rlc = chunkp.tile([128, CHUNK], bf16, tag="rlc")
                                    if do_act:
                                        nc.scalar.activation(
                                            out=rlc, in_=ldc,
                                            func=mybir.ActivationFunctionType.Relu,
                                            bias=bias_t, scale=1.0,
                                            accum_out=scr[:, ch : ch + 1],
                                        )
                                    if do_mm:
                                        rlc3, n_c = fold_c(rlc, XC, "foldc")
                                        for ci in range(n_c):
                                            nc.tensor.matmul(
                                                psum_t[:, ch * XC:(ch + 1) * XC],
                                                ones, rlc3[:, :, ci],
                                                start=False,
                                                stop=(is_last_tile
                                                      and ch == N_CHUNKS - 1
                                                      and ci == n_c - 1),
                                                # accumulation onto the
                                                # memset-zeroed bank; the sim's
                                                # bank-granular group assert
                                                # can't express this
                                                skip_group_check=(
                                                    N_CHUNK_TILES
                                                    >= TILES_PER_SAMPLE),
                                            )
                                if do_act:
                                    nc.vector.reduce_max(
                                        rows_sb[:, k : k + 1], scr,
                                        axis=mybir.AxisListType.X)
                            else:
                                ld = loads.tile([128, FREE], f32)
                                dma_eng = (nc.scalar if (dma_alt and (k % 2)) else nc.sync)
                                dma_eng.dma_start(out=ld, in_=src[s * 4 + t])
                                rl = relus.tile([128, FREE], bf16)
                                if do_act:
                                    nc.scalar.activation(
                                        out=rl, in_=ld,
                                        func=mybir.ActivationFunctionType.Relu,
                                        bias=bias_t, scale=1.0,
                                        accum_out=rows_sb[:, k : k + 1],
                                    )
                                if do_mm:
                                    rl3, n_c = fold_c(rl, W, "fold")
                                    for ci in range(n_c):
                                        nc.tensor.matmul(
                                            psum_t, ones, rl3[:, :, ci],
                                            start=(t == 0 and ci == 0),
                                            stop=(not is_last_st
                                                  and t == TILES_PER_SAMPLE - 1
                                                  and ci == n_c - 1),
                                        )
                        if do_mm:
                            csb = colsb.tile([1, W], f32)
                            nc.vector.tensor_copy(csb, psum_t)
                            nc.sync.dma_start(out=cols[st : st + 1], in_=csb)
                if not do_mm:
                    csb = colsb.tile([4, W], f32)
                    nc.vector.memset(csb, 1.0)
                    nc.sync.dma_start(out=cols, in_=csb)
                if not do_act:
                    nc.vector.memset(rows_sb[:, :1], 1.0)
                nc.sync.dma_start(out=rows, in_=rows_sb)

            if repeat == 1:
                body()
            else:
                with tc.For_i(0, repeat, 1,
                              hint_engines=(mybir.EngineType.PE,)) as iv:
                    body(iv)

    nc.compile()
    return nc


STRIP_ROWS = (0, 1, 510, 511)            # edge rows read by the strip kernel
N_STRIP_ROWS = len(STRIP_ROWS)
N_ST = 2 * SAMPLES_PER_CORE              # 4 sample-tensors per core
STRIP_P = 8                              # SBUF partitions per strip row
STRIP_F = FREE // STRIP_P                # 1024 free elems per partition


def build_strip_nc(repeat: int = 1):
    """Phase-1 kernel: edge-row summaries only.

    Input  strips [128, 1024] f32: partition p = (st*4 + j)*8 + q holds
    elements [q*1024, (q+1)*1024) of edge row j of sample-tensor st
    (st: pred s0, pred s1, exp s0, exp s1; j indexes STRIP_ROWS).
    Output summ [128, 3] f32 per partition:
      [:,0] sum of relu(v - T) over the partition's 1024 elems (row chunk)
      [:,1] max of relu over free [0:16)      = column x=0   (valid at q=0)
      [:,2] max of relu over free [1008:1024) = column x=511 (valid at q=7)
    """
    nc = bacc.Bacc("TRN2", debug=False)
    strips = nc.dram_tensor("strips", [128, STRIP_F], f32,
                            kind="ExternalInput").ap()
    summ = nc.dram_tensor("summ", [128, 3], f32, kind="ExternalOutput").ap()

    with tile.TileContext(nc) as tc:
        with (
            tc.tile_pool(name="singles", bufs=1) as singles,
            tc.tile_pool(name="loads", bufs=3) as loads,
            tc.tile_pool(name="relus", bufs=3) as relus,
            tc.tile_pool(name="outs", bufs=3) as outs,
        ):
            bias_t = singles.tile([128, 1], f32)
            # partitions 0..63 are pred rows (T=0.3), 64..127 exp (T=0.5)
            nc.vector.memset(bias_t[0:64], -THRESHOLDS[0])
            nc.vector.memset(bias_t[64:128], -THRESHOLDS[1])

            def body(_iv=None):
                ld = loads.tile([128, STRIP_F], f32)
                nc.sync.dma_start(out=ld, in_=strips)
                rl = relus.tile([128, STRIP_F], bf16)
                sm = outs.tile([128, 3], f32)
                nc.scalar.activation(
                    out=rl, in_=ld,
                    func=mybir.ActivationFunctionType.Relu,
                    bias=bias_t, scale=1.0,
                    accum_out=sm[:, 0:1])
                nc.vector.reduce_max(sm[:, 1:2], rl[:, 0:C],
                                     axis=mybir.AxisListType.X)
                nc.vector.reduce_max(sm[:, 2:3], rl[:, STRIP_F - C:STRIP_F],
                                     axis=mybir.AxisListType.X)
                nc.sync.dma_start(out=summ, in_=sm)

            if repeat == 1:
                body()
            else:
                with tc.For_i(0, repeat, 1,
                              hint_engines=(mybir.EngineType.PE,)) as iv:
                    body(iv)

    nc.compile()
    return nc


def _shard_strips(p, e):
    """Host gather of the edge rows: [N_CORES] maps of strips [128, 1024]."""
    rows = np.asarray(STRIP_ROWS)
    p3 = p.reshape(B, H, FREE)
    e3 = e.reshape(B, H, FREE)
    maps = []
    for c in range(N_CORES):
        blocks = [arr[c * SAMPLES_PER_CORE + s][rows]
                  for arr in (p3, e3) for s in range(SAMPLES_PER_CORE)]
        strips = np.ascontiguousarray(
            np.concatenate(blocks, axis=0)).reshape(128, STRIP_F)
        maps.append({"strips": strips})
    return maps


def _combine_strip(results):
    """Host epilogue for phase 1. Returns the scalar result, or None if any
    bbox corner is not provable from the edge strips (-> full fallback)."""
    f = np.float32
    penalties = []
    for core in range(N_CORES):
        sm = results[core]["summ"]  # [128, 3]
        boxes = []
        for st in range(N_ST):
            base = st * N_STRIP_ROWS * STRIP_P
            row_any = [
                bool((sm[base + j * STRIP_P: base + (j + 1) * STRIP_P, 0]
                      > 0).any())
                for j in range(N_STRIP_ROWS)]
            col0_any = any(sm[base + j * STRIP_P, 1] > 0
                           for j in range(N_STRIP_ROWS))
            col511_any = any(sm[base + j * STRIP_P + STRIP_P - 1, 2] > 0
                             for j in range(N_STRIP_ROWS))
            if row_any[0]:
                y1 = STRIP_ROWS[0]
            elif row_any[1]:
                y1 = STRIP_ROWS[1]
            else:
                return None  # y_min not determined by the top strip
            if row_any[3]:
                y2 = STRIP_ROWS[3]
            elif row_any[2]:
                y2 = STRIP_ROWS[2]
            else:
                return None
            if not col0_any or not col511_any:
                return None  # x extremes not provable from strips
            boxes.append((y1, 0, y2, W - 1))
        for s in range(SAMPLES_PER_CORE):
            py1, px1, py2, px2 = boxes[s]                      # pred
            ty1, tx1, ty2, tx2 = boxes[SAMPLES_PER_CORE + s]   # exp
            pred_area = f((py2 - py1 + 1) * (px2 - px1 + 1))
            true_area = f((ty2 - ty1 + 1) * (tx2 - tx1 + 1))
            area_penalty = f(max(f(0.0), f(pred_area - true_area))) / f(true_area + f(1.0))
            pcy, pcx = f(py1 + py2) / f(2.0), f(px1 + px2) / f(2.0)
            tcy, tcx = f(ty1 + ty2) / f(2.0), f(tx1 + tx2) / f(2.0)
            center_offset = np.sqrt(np.square(f(pcy - tcy)) + np.square(f(pcx - tcx))) / f(20.0)
            penalties.append(f(area_penalty + center_offset))
    mean = np.mean(np.asarray(penalties, dtype=np.float32), dtype=np.float32)
    return np.asarray(np.float32(PENALTY_WEIGHT) * mean, dtype=np.float32)


def _shard_inputs(prediction_probs, expected_onehot):
    p = np.ascontiguousarray(np.asarray(prediction_probs), dtype=np.float32)
    e = np.ascontiguousarray(np.asarray(expected_onehot), dtype=np.float32)
    p = p.reshape(N_CORES, SAMPLES_PER_CORE * TILES_PER_SAMPLE, 128, FREE)
    e = e.reshape(N_CORES, SAMPLES_PER_CORE * TILES_PER_SAMPLE, 128, FREE)
    return [{"pred": p[c], "exp": e[c]} for c in range(N_CORES)]


def _bbox_from_any(row_any, col_any):
    ys = np.nonzero(row_any)[0]
    xs = np.nonzero(col_any)[0]
    if ys.size == 0:
        return 0, 0, 1, 1
    return int(ys[0]), int(xs[0]), int(ys[-1]), int(xs[-1])


def _combine(results):
    """Host epilogue: exact bbox/penalty math from row/col summaries."""
    f = np.float32
    penalties = []
    for core in range(N_CORES):
        rows = results[core]["rows"]  # [128, 16]
        cols = results[core]["cols"]  # [4, 512]
        for s in range(SAMPLES_PER_CORE):
            boxes = []
            for tensor_idx in range(2):
                k0 = tensor_idx * 8 + s * 4
                row_any = rows[:, k0 : k0 + 4].T.ravel() > 0  # y = t*128 + p
                col_any = cols[tensor_idx * SAMPLES_PER_CORE + s] > 0
                boxes.append(_bbox_from_any(row_any, col_any))
            (py1, px1, py2, px2), (ty1, tx1, ty2, tx2) = boxes
            pred_area = f((py2 - py1 + 1) * (px2 - px1 + 1))
            true_area = f((ty2 - ty1 + 1) * (tx2 - tx1 + 1))
            area_penalty = f(max(f(0.0), f(pred_area - true_area))) / f(true_area + f(1.0))
            pcy, pcx = f(py1 + py2) / f(2.0), f(px1 + px2) / f(2.0)
            tcy, tcx = f(ty1 + ty2) / f(2.0), f(tx1 + tx2) / f(2.0)
            center_offset = np.sqrt(np.square(f(pcy - tcy)) + np.square(f(pcx - tcx))) / f(20.0)
            penalties.append(f(area_penalty + center_offset))
    mean = np.mean(np.asarray(penalties, dtype=np.float32), dtype=np.float32)
    return np.asarray(np.float32(PENALTY_WEIGHT) * mean, dtype=np.float32)


_NC_CACHE = {}


def _run_spmd(nc, in_maps):
    last_exc = None
    for attempt in range(3):  # the axon device occasionally flakes transiently
        try:
            return run_bass_kernel_spmd(nc, in_maps, core_ids=list(range(N_CORES)))
        except Exception as e:  # noqa: BLE001
            last_exc = e
            try:
                # an NRT_EXEC_UNIT_UNRECOVERABLE poisons the PJRT mesh for
                # the whole process; dropping the backend forces a reconnect
                import jax.extend.backend

                jax.extend.backend.clear_backends()
            except Exception:  # noqa: BLE001
                pass
            time.sleep(5.0)
    raise last_exc


def kernel(prediction_probs, expected_onehot):
    p = np.ascontiguousarray(np.asarray(prediction_probs), dtype=np.float32)
    e = np.ascontiguousarray(np.asarray(expected_onehot), dtype=np.float32)

    # Phase 1: edge-strip kernel (reads 4 MiB total instead of 512 MiB).
    if "strip" not in _NC_CACHE:
        _NC_CACHE["strip"] = build_strip_nc()
    res = _run_spmd(_NC_CACHE["strip"], _shard_strips(p, e))
    out = _combine_strip(res.results)
    if out is not None:
        return out

    # Phase 2 (exact fallback for unresolved corners): full-read kernel.
    if "nc" not in _NC_CACHE:
        _NC_CACHE["nc"] = build_nc()
    res = _run_spmd(_NC_CACHE["nc"], _shard_inputs(p, e))
    return _combine(res.results)



# revision 7
# speedup vs baseline: 20.8304x; 1.0868x over previous
"""Trainium2 Bass kernel for nn_BoundingBoxDiscipline.

Computes PENALTY_WEIGHT * mean_B(area_penalty + center_offset) where the
penalties are derived from per-sample bounding boxes of thresholded masks:
    pred_mask = max_C(prediction_probs) > 0.3
    true_mask = max_C(expected_onehot)  > 0.5

The result depends ONLY on the bbox corners: the first/last nonempty row
and whether columns 0 / 511 are nonempty. That admits an exact adaptive
two-phase algorithm:

Phase 1 (strip kernel, the fast path): each core reads only the 4 edge
rows {0, 1, 510, 511} of its 4 sample-tensors (512 KiB/core instead of
64 MiB/core) as one contiguous DMA, applies relu(v - T) on ScalarE with
accum_out giving per-row-chunk sums, and reduces the 16-channel blocks of
columns 0 and 511 on VectorE. On the host, for each sample-tensor:
  - row 0 (or 1) nonempty   -> y_min proven exactly;
  - row 511 (or 510)        -> y_max proven exactly;
  - a true pixel at x=0 in any strip row   -> x_min = 0 proven;
  - a true pixel at x=511 in any strip row -> x_max = 511 proven.
All comparisons are exact (v > T <=> relu(v - T) > 0 in fp32/bf16, and
sums/maxes of non-negative values are > 0 iff some element is > 0).

Phase 2 (full kernel, fallback): if ANY corner fails to resolve from the
strips (possible only for nearly-empty/adversarial masks, probability
~1e-9 per sample for random dense inputs), rerun with the full-read
kernel, which computes complete row/col summaries from all 512 MiB and is
exact for every input. The combined algorithm is therefore exact for all
inputs while reading 128x less data on typical ones.

Full-kernel device plan, data-parallel over batch (2 samples/core):
  - DMA (the roofline: 64 MiB/core at ~342 GB/s measured): 16 tiles of
    [128 y-rows, 8192 (x*16+c)] fp32 per core.
  - ScalarE: relu(v - T) -> bf16 tile, with accum_out giving the per-row
    (free-axis) sum in one pass -> row_any.
  - VectorE: pairwise max folds channels 16 -> 4 (positivity-preserving),
    keeping the cold-clocked TensorE off the critical path.
  - TensorE: ones[128,1].T @ folded[:, :, c] accumulated over 4 y-tiles
    and 4 channel views into one PSUM [1, 512] -> col sums.
Measured 201.7-202.9 us/iter steady-state vs a 196.2 us DMA-only floor.
"""

import time

import numpy as np

import concourse.bacc as bacc
import concourse.tile as tile
from concourse import mybir
from concourse.bass_utils import run_bass_kernel_spmd

N_CORES = 8
B, H, W, C = 16, 512, 512, 16
SAMPLES_PER_CORE = B // N_CORES          # 2
TILES_PER_SAMPLE = H // 128              # 4
FREE = W * C                             # 8192
THRESHOLDS = (0.3, 0.5)                  # (prediction_probs, expected_onehot)
PENALTY_WEIGHT = 0.05

f32 = mybir.dt.float32
bf16 = mybir.dt.bfloat16


def build_nc(repeat: int = 1, do_act: bool = True, do_mm: bool = True,
             c_fold: int = 4, dma_alt: bool = False):
    """Build the per-core Bass module. `repeat` wraps the body in a device
    loop; `do_act`/`do_mm` exist only for ablation timing experiments.
    `c_fold` is the channel count after DVE pairwise max-folding (16 = no
    fold); folding moves colsum work off the cold-clocked TensorE onto the
    otherwise-idle DVE. The graded path uses the defaults."""
    assert c_fold in (2, 4, 8, 16)
    nc = bacc.Bacc("TRN2", debug=False)

    n_tiles = 2 * SAMPLES_PER_CORE * TILES_PER_SAMPLE  # 16 (tensor, sample, ytile)
    n_st = 2 * SAMPLES_PER_CORE                        # 4 sample-tensors

    pred = nc.dram_tensor(
        "pred", [SAMPLES_PER_CORE * TILES_PER_SAMPLE, 128, FREE], f32,
        kind="ExternalInput").ap()
    exp = nc.dram_tensor(
        "exp", [SAMPLES_PER_CORE * TILES_PER_SAMPLE, 128, FREE], f32,
        kind="ExternalInput").ap()
    rows = nc.dram_tensor("rows", [128, n_tiles], f32, kind="ExternalOutput").ap()
    cols = nc.dram_tensor("cols", [n_st, W], f32, kind="ExternalOutput").ap()

    with tile.TileContext(nc) as tc:
        with (
            tc.tile_pool(name="singles", bufs=1) as singles,
            tc.tile_pool(name="loads", bufs=2) as loads,
            tc.tile_pool(name="relus", bufs=2) as relus,
            tc.tile_pool(name="rowsp", bufs=1) as rowsp,
            tc.tile_pool(name="chunkp", bufs=8) as chunkp,
            tc.tile_pool(name="colsb", bufs=2) as colsb,
            tc.tile_pool(name="psum", bufs=2, space="PSUM") as psum,
        ):
            ones = singles.tile([128, 1], bf16)
            nc.vector.memset(ones, 1.0)
            biases = []
            for thr in THRESHOLDS:
                bias_t = singles.tile([128, 1], f32, tag=f"bias{thr}")
                nc.vector.memset(bias_t, -thr)
                biases.append(bias_t)
            rows_sb = rowsp.tile([128, n_tiles], f32)

            def fold_c(rl_flat, n_x, tag):
                """DVE pairwise max over channel halves: 16 -> c_fold chans.
                Positivity-preserving, so col_any is unchanged."""
                cur, cur_c = rl_flat, C
                while cur_c > c_fold:
                    nxt = cur_c // 2
                    out_t = relus.tile([128, n_x * nxt], bf16, tag=f"{tag}{nxt}")
                    cur3 = cur.rearrange("p (x c) -> p x c", c=cur_c)
                    out3 = out_t.rearrange("p (x c) -> p x c", c=nxt)
                    nc.vector.tensor_tensor(
                        out=out3, in0=cur3[:, :, 0:nxt], in1=cur3[:, :, nxt:cur_c],
                        op=mybir.AluOpType.max)
                    cur, cur_c = out_t, nxt
                return cur.rearrange("p (x c) -> p x c", c=cur_c), cur_c

            def body(_iv=None):
                # The last two tiles are processed in 4 free-dim chunks each:
                # every trailing ACT op is then ~1.9 us < the 2.9 us chunk DMA
                # cadence, so ScalarE never backlogs the tail and the
                # post-final-DMA compute is ~2 us instead of ~13 us.
                N_CHUNKS = 8
                N_CHUNK_TILES = 2
                CHUNK = FREE // N_CHUNKS          # 1024 free elems = 64 x's
                XC = CHUNK // C                   # 128
                last_scratch = rowsp.tile([128, N_CHUNK_TILES * N_CHUNKS], f32)

                for tensor_idx, src in ((0, pred), (1, exp)):
                    bias_t = biases[tensor_idx]
                    for s in range(SAMPLES_PER_CORE):
                        st = tensor_idx * SAMPLES_PER_CORE + s
                        is_last_st = st == 2 * SAMPLES_PER_CORE - 1
                        psum_t = psum.tile([1, W], f32)
                        if is_last_st and N_CHUNK_TILES >= TILES_PER_SAMPLE and do_mm:
                            # all tiles chunked -> no N=512 start=True matmul
                            # to clear the bank; zero it and accumulate onto
                            # zeros (correct under any has_written semantics)
                            nc.vector.memset(psum_t, 0.0)
                        for t in range(TILES_PER_SAMPLE):
                            k = tensor_idx * 8 + s * 4 + t
                            if is_last_st and t >= TILES_PER_SAMPLE - N_CHUNK_TILES:
                                ct = t - (TILES_PER_SAMPLE - N_CHUNK_TILES)
                                scr = last_scratch[:, ct * N_CHUNKS:(ct + 1) * N_CHUNKS]
                                is_last_tile = t == TILES_PER_SAMPLE - 1
                                for ch in range(N_CHUNKS):
                                    ldc = chunkp.tile([128, CHUNK], f32, tag="ldc")
                                    nc.sync.dma_start(
                                        out=ldc,
                                        in_=src[s * 4 + t, :,
                                                ch * CHUNK:(ch + 1) * CHUNK])
                                    rlc = chunkp.tile([128, CHUNK], bf16, tag="rlc")
                                    if do_act:
                                        nc.scalar.activation(
                                            out=rlc, in_=ldc,
                                            func=mybir.ActivationFunctionType.Relu,
                                            bias=bias_t, scale=1.0,
                                            accum_out=scr[:, ch : ch + 1],
                                        )
                                    if do_mm:
                                        rlc3, n_c = fold_c(rlc, XC, "foldc")
                                        for ci in range(n_c):
                                            nc.tensor.matmul(
                                                psum_t[:, ch * XC:(ch + 1) * XC],
                                                ones, rlc3[:, :, ci],
                                                start=False,
                                                stop=(is_last_tile
                                                      and ch == N_CHUNKS - 1
                                                      and ci == n_c - 1),
                                                # accumulation onto the
                                                # memset-zeroed bank; the sim's
                                                # bank-granular group assert
                                                # can't express this
                                                skip_group_check=(
                                                    N_CHUNK_TILES
                                                    >= TILES_PER_SAMPLE),
                                            )
                                if do_act:
                                    nc.vector.reduce_max(
                                        rows_sb[:, k : k + 1], scr,
                                        axis=mybir.AxisListType.X)
                            else:
                                ld = loads.tile([128, FREE], f32)
                                dma_eng = (nc.scalar if (dma_alt and (k % 2)) else nc.sync)
                                dma_eng.dma_start(out=ld, in_=src[s * 4 + t])
                                rl = relus.tile([128, FREE], bf16)
                                if do_act:
                                    nc.scalar.activation(
                                        out=rl, in_=ld,
                                        func=mybir.ActivationFunctionType.Relu,
                                        bias=bias_t, scale=1.0,
                                        accum_out=rows_sb[:, k : k + 1],
                                    )
                                if do_mm:
                                    rl3, n_c = fold_c(rl, W, "fold")
                                    for ci in range(n_c):
                                        nc.tensor.matmul(
                                            psum_t, ones, rl3[:, :, ci],
                                            start=(t == 0 and ci == 0),
                                            stop=(not is_last_st
                                                  and t == TILES_PER_SAMPLE - 1
                                                  and ci == n_c - 1),
                                        )
                        if do_mm:
                            csb = colsb.tile([1, W], f32)
                            nc.vector.tensor_copy(csb, psum_t)
                            nc.sync.dma_start(out=cols[st : st + 1], in_=csb)
                if not do_mm:
                    csb = colsb.tile([4, W], f32)
                    nc.vector.memset(csb, 1.0)
                    nc.sync.dma_start(out=cols, in_=csb)
                if not do_act:
                    nc.vector.memset(rows_sb[:, :1], 1.0)
                nc.sync.dma_start(out=rows, in_=rows_sb)

            if repeat == 1:
                body()
            else:
                with tc.For_i(0, repeat, 1,
                              hint_engines=(mybir.EngineType.PE,)) as iv:
                    body(iv)

    nc.compile()
    return nc


STRIP_ROWS = (0, 1, 510, 511)            # edge rows read by the strip kernel
N_STRIP_ROWS = len(STRIP_ROWS)
N_ST = 2 * SAMPLES_PER_CORE              # 4 sample-tensors per core
STRIP_P = 8                              # SBUF partitions per strip row
STRIP_F = FREE // STRIP_P                # 1024 free elems per partition


def build_strip_nc(repeat: int = 1):
    """Phase-1 kernel: edge-row summaries only.

    Input  strips [128, 1024] f32: partition p = (st*4 + j)*8 + q holds
    elements [q*1024, (q+1)*1024) of edge row j of sample-tensor st
    (st: pred s0, pred s1, exp s0, exp s1; j indexes STRIP_ROWS).
    Output summ [128, 3] f32 per partition, raw maxes (host compares vs T):
      [:,0] max of v over the partition's 1024 elems (row chunk)
      [:,1] max of v over free [0:16)      = column x=0   (valid at q=0)
      [:,2] max of v over free [1008:1024) = column x=511 (valid at q=7)
    The in-DMA streams on the sync HWDGE queue while the tiny out-DMA
    rides the scalar queue, so back-to-back iterations overlap instead of
    serializing behind the compute -> out chain.
    """
    nc = bacc.Bacc("TRN2", debug=False)
    strips = nc.dram_tensor("strips", [128, STRIP_F], f32,
                            kind="ExternalInput").ap()
    summ = nc.dram_tensor("summ", [128, 3], f32, kind="ExternalOutput").ap()

    with tile.TileContext(nc) as tc:
        with (
            tc.tile_pool(name="loads", bufs=3) as loads,
            tc.tile_pool(name="outs", bufs=3) as outs,
        ):
            def body(_iv=None):
                ld = loads.tile([128, STRIP_F], f32)
                nc.sync.dma_start(out=ld, in_=strips)
                sm = outs.tile([128, 3], f32)
                nc.vector.reduce_max(sm[:, 0:1], ld,
                                     axis=mybir.AxisListType.X)
                nc.vector.reduce_max(sm[:, 1:2], ld[:, 0:C],
                                     axis=mybir.AxisListType.X)
                nc.vector.reduce_max(sm[:, 2:3], ld[:, STRIP_F - C:STRIP_F],
                                     axis=mybir.AxisListType.X)
                nc.scalar.dma_start(out=summ, in_=sm)

            if repeat == 1:
                body()
            else:
                with tc.For_i(0, repeat, 1,
                              hint_engines=(mybir.EngineType.PE,)) as iv:
                    body(iv)

    nc.compile()
    return nc


def _shard_strips(p, e):
    """Host gather of the edge rows: [N_CORES] maps of strips [128, 1024]."""
    rows = np.asarray(STRIP_ROWS)
    p3 = p.reshape(B, H, FREE)
    e3 = e.reshape(B, H, FREE)
    maps = []
    for c in range(N_CORES):
        blocks = [arr[c * SAMPLES_PER_CORE + s][rows]
                  for arr in (p3, e3) for s in range(SAMPLES_PER_CORE)]
        strips = np.ascontiguousarray(
            np.concatenate(blocks, axis=0)).reshape(128, STRIP_F)
        maps.append({"strips": strips})
    return maps


def _combine_strip(results):
    """Host epilogue for phase 1. Returns the scalar result, or None if any
    bbox corner is not provable from the edge strips (-> full fallback)."""
    f = np.float32
    penalties = []
    for core in range(N_CORES):
        sm = results[core]["summ"]  # [128, 3]
        boxes = []
        for st in range(N_ST):
            # fp32 compare, matching the reference's weak-typed `> 0.3`
            thr = np.float32(
                THRESHOLDS[0] if st < SAMPLES_PER_CORE else THRESHOLDS[1])
            base = st * N_STRIP_ROWS * STRIP_P
            row_any = [
                bool((sm[base + j * STRIP_P: base + (j + 1) * STRIP_P, 0]
                      > thr).any())
                for j in range(N_STRIP_ROWS)]
            col0_any = any(sm[base + j * STRIP_P, 1] > thr
                           for j in range(N_STRIP_ROWS))
            col511_any = any(sm[base + j * STRIP_P + STRIP_P - 1, 2] > thr
                             for j in range(N_STRIP_ROWS))
            if row_any[0]:
                y1 = STRIP_ROWS[0]
            elif row_any[1]:
                y1 = STRIP_ROWS[1]
            else:
                return None  # y_min not determined by the top strip
            if row_any[3]:
                y2 = STRIP_ROWS[3]
            elif row_any[2]:
                y2 = STRIP_ROWS[2]
            else:
                return None
            if not col0_any or not col511_any:
                return None  # x extremes not provable from strips
            boxes.append((y1, 0, y2, W - 1))
        for s in range(SAMPLES_PER_CORE):
            py1, px1, py2, px2 = boxes[s]                      # pred
            ty1, tx1, ty2, tx2 = boxes[SAMPLES_PER_CORE + s]   # exp
            pred_area = f((py2 - py1 + 1) * (px2 - px1 + 1))
            true_area = f((ty2 - ty1 + 1) * (tx2 - tx1 + 1))
            area_penalty = f(max(f(0.0), f(pred_area - true_area))) / f(true_area + f(1.0))
            pcy, pcx = f(py1 + py2) / f(2.0), f(px1 + px2) / f(2.0)
            tcy, tcx = f(ty1 + ty2) / f(2.0), f(tx1 + tx2) / f(2.0)
            center_offset = np.sqrt(np.square(f(pcy - tcy)) + np.square(f(pcx - tcx))) / f(20.0)
            penalties.append(f(area_penalty + center_offset))
    mean = np.mean(np.asarray(penalties, dtype=np.float32), dtype=np.float32)
    return np.asarray(np.float32(PENALTY_WEIGHT) * mean, dtype=np.float32)


def _shard_inputs(prediction_probs, expected_onehot):
    p = np.ascontiguousarray(np.asarray(prediction_probs), dtype=np.float32)
    e = np.ascontiguousarray(np.asarray(expected_onehot), dtype=np.float32)
    p = p.reshape(N_CORES, SAMPLES_PER_CORE * TILES_PER_SAMPLE, 128, FREE)
    e = e.reshape(N_CORES, SAMPLES_PER_CORE * TILES_PER_SAMPLE, 128, FREE)
    return [{"pred": p[c], "exp": e[c]} for c in range(N_CORES)]


def _bbox_from_any(row_any, col_any):
    ys = np.nonzero(row_any)[0]
    xs = np.nonzero(col_any)[0]
    if ys.size == 0:
        return 0, 0, 1, 1
    return int(ys[0]), int(xs[0]), int(ys[-1]), int(xs[-1])


def _combine(results):
    """Host epilogue: exact bbox/penalty math from row/col summaries."""
    f = np.float32
    penalties = []
    for core in range(N_CORES):
        rows = results[core]["rows"]  # [128, 16]
        cols = results[core]["cols"]  # [4, 512]
        for s in range(SAMPLES_PER_CORE):
            boxes = []
            for tensor_idx in range(2):
                k0 = tensor_idx * 8 + s * 4
                row_any = rows[:, k0 : k0 + 4].T.ravel() > 0  # y = t*128 + p
                col_any = cols[tensor_idx * SAMPLES_PER_CORE + s] > 0
                boxes.append(_bbox_from_any(row_any, col_any))
            (py1, px1, py2, px2), (ty1, tx1, ty2, tx2) = boxes
            pred_area = f((py2 - py1 + 1) * (px2 - px1 + 1))
            true_area = f((ty2 - ty1 + 1) * (tx2 - tx1 + 1))
            area_penalty = f(max(f(0.0), f(pred_area - true_area))) / f(true_area + f(1.0))
            pcy, pcx = f(py1 + py2) / f(2.0), f(px1 + px2) / f(2.0)
            tcy, tcx = f(ty1 + ty2) / f(2.0), f(tx1 + tx2) / f(2.0)
            center_offset = np.sqrt(np.square(f(pcy - tcy)) + np.square(f(pcx - tcx))) / f(20.0)
            penalties.append(f(area_penalty + center_offset))
    mean = np.mean(np.asarray(penalties, dtype=np.float32), dtype=np.float32)
    return np.asarray(np.float32(PENALTY_WEIGHT) * mean, dtype=np.float32)


_NC_CACHE = {}


def _run_spmd(nc, in_maps):
    last_exc = None
    for attempt in range(3):  # the axon device occasionally flakes transiently
        try:
            return run_bass_kernel_spmd(nc, in_maps, core_ids=list(range(N_CORES)))
        except Exception as e:  # noqa: BLE001
            last_exc = e
            try:
                # an NRT_EXEC_UNIT_UNRECOVERABLE poisons the PJRT mesh for
                # the whole process; dropping the backend forces a reconnect
                import jax.extend.backend

                jax.extend.backend.clear_backends()
            except Exception:  # noqa: BLE001
                pass
            time.sleep(5.0)
    raise last_exc


def kernel(prediction_probs, expected_onehot):
    p = np.ascontiguousarray(np.asarray(prediction_probs), dtype=np.float32)
    e = np.ascontiguousarray(np.asarray(expected_onehot), dtype=np.float32)

    # Phase 1: edge-strip kernel (reads 4 MiB total instead of 512 MiB).
    if "strip" not in _NC_CACHE:
        _NC_CACHE["strip"] = build_strip_nc()
    res = _run_spmd(_NC_CACHE["strip"], _shard_strips(p, e))
    out = _combine_strip(res.results)
    if out is not None:
        return out

    # Phase 2 (exact fallback for unresolved corners): full-read kernel.
    if "nc" not in _NC_CACHE:
        _NC_CACHE["nc"] = build_nc()
    res = _run_spmd(_NC_CACHE["nc"], _shard_inputs(p, e))
    return _combine(res.results)



# revision 12
# speedup vs baseline: 82.3971x; 3.9556x over previous
"""Trainium2 Bass kernel for nn_BoundingBoxDiscipline.

Computes PENALTY_WEIGHT * mean_B(area_penalty + center_offset) where the
penalties are derived from per-sample bounding boxes of thresholded masks:
    pred_mask = max_C(prediction_probs) > 0.3
    true_mask = max_C(expected_onehot)  > 0.5

The result depends ONLY on the bbox corners: the first/last nonempty row
and whether columns 0 / 511 are nonempty. That admits an exact adaptive
two-phase algorithm:

Phase 1 (strip kernel, the fast path): each core reads only the 4 edge
rows {0, 1, 510, 511} of its 4 sample-tensors (512 KiB/core instead of
64 MiB/core) as one contiguous DMA, applies relu(v - T) on ScalarE with
accum_out giving per-row-chunk sums, and reduces the 16-channel blocks of
columns 0 and 511 on VectorE. On the host, for each sample-tensor:
  - row 0 (or 1) nonempty   -> y_min proven exactly;
  - row 511 (or 510)        -> y_max proven exactly;
  - a true pixel at x=0 in any strip row   -> x_min = 0 proven;
  - a true pixel at x=511 in any strip row -> x_max = 511 proven.
All comparisons are exact (v > T <=> relu(v - T) > 0 in fp32/bf16, and
sums/maxes of non-negative values are > 0 iff some element is > 0).

Phase 2 (full kernel, fallback): if ANY corner fails to resolve from the
strips (possible only for nearly-empty/adversarial masks, probability
~1e-9 per sample for random dense inputs), rerun with the full-read
kernel, which computes complete row/col summaries from all 512 MiB and is
exact for every input. The combined algorithm is therefore exact for all
inputs while reading 128x less data on typical ones.

Full-kernel device plan, data-parallel over batch (2 samples/core):
  - DMA (the roofline: 64 MiB/core at ~342 GB/s measured): 16 tiles of
    [128 y-rows, 8192 (x*16+c)] fp32 per core.
  - ScalarE: relu(v - T) -> bf16 tile, with accum_out giving the per-row
    (free-axis) sum in one pass -> row_any.
  - VectorE: pairwise max folds channels 16 -> 4 (positivity-preserving),
    keeping the cold-clocked TensorE off the critical path.
  - TensorE: ones[128,1].T @ folded[:, :, c] accumulated over 4 y-tiles
    and 4 channel views into one PSUM [1, 512] -> col sums.
Measured 201.7-202.9 us/iter steady-state vs a 196.2 us DMA-only floor.
"""

import time

import numpy as np

import concourse.bacc as bacc
import concourse.tile as tile
from concourse import mybir
from concourse.bass_utils import run_bass_kernel_spmd

N_CORES = 8
B, H, W, C = 16, 512, 512, 16
SAMPLES_PER_CORE = B // N_CORES          # 2
TILES_PER_SAMPLE = H // 128              # 4
FREE = W * C                             # 8192
THRESHOLDS = (0.3, 0.5)                  # (prediction_probs, expected_onehot)
PENALTY_WEIGHT = 0.05

f32 = mybir.dt.float32
bf16 = mybir.dt.bfloat16


def build_nc(repeat: int = 1, do_act: bool = True, do_mm: bool = True,
             c_fold: int = 4, dma_alt: bool = False):
    """Build the per-core Bass module. `repeat` wraps the body in a device
    loop; `do_act`/`do_mm` exist only for ablation timing experiments.
    `c_fold` is the channel count after DVE pairwise max-folding (16 = no
    fold); folding moves colsum work off the cold-clocked TensorE onto the
    otherwise-idle DVE. The graded path uses the defaults."""
    assert c_fold in (2, 4, 8, 16)
    nc = bacc.Bacc("TRN2", debug=False)

    n_tiles = 2 * SAMPLES_PER_CORE * TILES_PER_SAMPLE  # 16 (tensor, sample, ytile)
    n_st = 2 * SAMPLES_PER_CORE                        # 4 sample-tensors

    pred = nc.dram_tensor(
        "pred", [SAMPLES_PER_CORE * TILES_PER_SAMPLE, 128, FREE], f32,
        kind="ExternalInput").ap()
    exp = nc.dram_tensor(
        "exp", [SAMPLES_PER_CORE * TILES_PER_SAMPLE, 128, FREE], f32,
        kind="ExternalInput").ap()
    rows = nc.dram_tensor("rows", [128, n_tiles], f32, kind="ExternalOutput").ap()
    cols = nc.dram_tensor("cols", [n_st, W], f32, kind="ExternalOutput").ap()

    with tile.TileContext(nc) as tc:
        with (
            tc.tile_pool(name="singles", bufs=1) as singles,
            tc.tile_pool(name="loads", bufs=2) as loads,
            tc.tile_pool(name="relus", bufs=2) as relus,
            tc.tile_pool(name="rowsp", bufs=1) as rowsp,
            tc.tile_pool(name="chunkp", bufs=8) as chunkp,
            tc.tile_pool(name="colsb", bufs=2) as colsb,
            tc.tile_pool(name="psum", bufs=2, space="PSUM") as psum,
        ):
            ones = singles.tile([128, 1], bf16)
            nc.vector.memset(ones, 1.0)
            biases = []
            for thr in THRESHOLDS:
                bias_t = singles.tile([128, 1], f32, tag=f"bias{thr}")
                nc.vector.memset(bias_t, -thr)
                biases.append(bias_t)
            rows_sb = rowsp.tile([128, n_tiles], f32)

            def fold_c(rl_flat, n_x, tag):
                """DVE pairwise max over channel halves: 16 -> c_fold chans.
                Positivity-preserving, so col_any is unchanged."""
                cur, cur_c = rl_flat, C
                while cur_c > c_fold:
                    nxt = cur_c // 2
                    out_t = relus.tile([128, n_x * nxt], bf16, tag=f"{tag}{nxt}")
                    cur3 = cur.rearrange("p (x c) -> p x c", c=cur_c)
                    out3 = out_t.rearrange("p (x c) -> p x c", c=nxt)
                    nc.vector.tensor_tensor(
                        out=out3, in0=cur3[:, :, 0:nxt], in1=cur3[:, :, nxt:cur_c],
                        op=mybir.AluOpType.max)
                    cur, cur_c = out_t, nxt
                return cur.rearrange("p (x c) -> p x c", c=cur_c), cur_c

            def body(_iv=None):
                # The last two tiles are processed in 4 free-dim chunks each:
                # every trailing ACT op is then ~1.9 us < the 2.9 us chunk DMA
                # cadence, so ScalarE never backlogs the tail and the
                # post-final-DMA compute is ~2 us instead of ~13 us.
                N_CHUNKS = 8
                N_CHUNK_TILES = 2
                CHUNK = FREE // N_CHUNKS          # 1024 free elems = 64 x's
                XC = CHUNK // C                   # 128
                last_scratch = rowsp.tile([128, N_CHUNK_TILES * N_CHUNKS], f32)

                for tensor_idx, src in ((0, pred), (1, exp)):
                    bias_t = biases[tensor_idx]
                    for s in range(SAMPLES_PER_CORE):
                        st = tensor_idx * SAMPLES_PER_CORE + s
                        is_last_st = st == 2 * SAMPLES_PER_CORE - 1
                        psum_t = psum.tile([1, W], f32)
                        if is_last_st and N_CHUNK_TILES >= TILES_PER_SAMPLE and do_mm:
                            # all tiles chunked -> no N=512 start=True matmul
                            # to clear the bank; zero it and accumulate onto
                            # zeros (correct under any has_written semantics)
                            nc.vector.memset(psum_t, 0.0)
                        for t in range(TILES_PER_SAMPLE):
                            k = tensor_idx * 8 + s * 4 + t
                            if is_last_st and t >= TILES_PER_SAMPLE - N_CHUNK_TILES:
                                ct = t - (TILES_PER_SAMPLE - N_CHUNK_TILES)
                                scr = last_scratch[:, ct * N_CHUNKS:(ct + 1) * N_CHUNKS]
                                is_last_tile = t == TILES_PER_SAMPLE - 1
                                for ch in range(N_CHUNKS):
                                    ldc = chunkp.tile([128, CHUNK], f32, tag="ldc")
                                    nc.sync.dma_start(
                                        out=ldc,
                                        in_=src[s * 4 + t, :,
                                                ch * CHUNK:(ch + 1) * CHUNK])
                                    rlc = chunkp.tile([128, CHUNK], bf16, tag="rlc")
                                    if do_act:
                                        nc.scalar.activation(
                                            out=rlc, in_=ldc,
                                            func=mybir.ActivationFunctionType.Relu,
                                            bias=bias_t, scale=1.0,
                                            accum_out=scr[:, ch : ch + 1],
                                        )
                                    if do_mm:
                                        rlc3, n_c = fold_c(rlc, XC, "foldc")
                                        for ci in range(n_c):
                                            nc.tensor.matmul(
                                                psum_t[:, ch * XC:(ch + 1) * XC],
                                                ones, rlc3[:, :, ci],
                                                start=False,
                                                stop=(is_last_tile
                                                      and ch == N_CHUNKS - 1
                                                      and ci == n_c - 1),
                                                # accumulation onto the
                                                # memset-zeroed bank; the sim's
                                                # bank-granular group assert
                                                # can't express this
                                                skip_group_check=(
                                                    N_CHUNK_TILES
                                                    >= TILES_PER_SAMPLE),
                                            )
                                if do_act:
                                    nc.vector.reduce_max(
                                        rows_sb[:, k : k + 1], scr,
                                        axis=mybir.AxisListType.X)
                            else:
                                ld = loads.tile([128, FREE], f32)
                                dma_eng = (nc.scalar if (dma_alt and (k % 2)) else nc.sync)
                                dma_eng.dma_start(out=ld, in_=src[s * 4 + t])
                                rl = relus.tile([128, FREE], bf16)
                                if do_act:
                                    nc.scalar.activation(
                                        out=rl, in_=ld,
                                        func=mybir.ActivationFunctionType.Relu,
                                        bias=bias_t, scale=1.0,
                                        accum_out=rows_sb[:, k : k + 1],
                                    )
                                if do_mm:
                                    rl3, n_c = fold_c(rl, W, "fold")
                                    for ci in range(n_c):
                                        nc.tensor.matmul(
                                            psum_t, ones, rl3[:, :, ci],
                                            start=(t == 0 and ci == 0),
                                            stop=(not is_last_st
                                                  and t == TILES_PER_SAMPLE - 1
                                                  and ci == n_c - 1),
                                        )
                        if do_mm:
                            csb = colsb.tile([1, W], f32)
                            nc.vector.tensor_copy(csb, psum_t)
                            nc.sync.dma_start(out=cols[st : st + 1], in_=csb)
                if not do_mm:
                    csb = colsb.tile([4, W], f32)
                    nc.vector.memset(csb, 1.0)
                    nc.sync.dma_start(out=cols, in_=csb)
                if not do_act:
                    nc.vector.memset(rows_sb[:, :1], 1.0)
                nc.sync.dma_start(out=rows, in_=rows_sb)

            if repeat == 1:
                body()
            else:
                with tc.For_i(0, repeat, 1,
                              hint_engines=(mybir.EngineType.PE,)) as iv:
                    body(iv)

    nc.compile()
    return nc


STRIP_ROWS = (0, 511)                    # edge rows read by the strip kernel
N_STRIP_ROWS = len(STRIP_ROWS)
N_ST = 2 * SAMPLES_PER_CORE              # 4 sample-tensors per core
N_SLICES = N_ST * N_STRIP_ROWS           # 8 row-slices per core
STRIP_P = 128 // N_SLICES                # 16 SBUF partitions per strip row
STRIP_F = FREE // STRIP_P                # 512 free elems per partition


def build_strip_nc(repeat: int = 1, unroll: int = 1):
    """Phase-1 kernel: edge-row summaries only.

    Input  strips [128, 512] f32: partition p = g*16 + q holds elements
    [q*512, (q+1)*512) of edge row j of sample-tensor st, where the
    row-slice g = st*2 + j (st: pred s0, pred s1, exp s0, exp s1;
    j indexes STRIP_ROWS = (0, 511)).

    Per execution: one 256 KiB in-DMA (sync queue); ScalarE relu(v - T)
    with per-partition bias (-0.3 pred / -0.5 exp partitions) -> bf16;
    DVE per-partition maxes (full row chunk + the two 16-channel corner
    blocks of columns 0/511) masked so corner stats survive only on the
    partitions that actually hold those columns; one PE matmul with a
    group-selector packs the 128 partitions' evidence into PSUM [3, 8]
    (positivity-exact: sums of non-negative relu evidence); a 3-line
    96 B out-DMA (scalar queue).  Host checks summ > 0.

    `unroll` emits that many complete executions per For_i trip to
    amortize the ~3.6 us all-engine loop barrier; `repeat` is the trip
    count (total executions = repeat * unroll).
    """
    nc = bacc.Bacc("TRN2", debug=False)
    strips = nc.dram_tensor("strips", [128, STRIP_F], f32,
                            kind="ExternalInput").ap()
    # [:, 0:8] group selector S[p, g] = 1 iff p belongs to slice g;
    # [:, 8:11] validity mask (stat 0 everywhere, stat 1 only at q=0
    # partitions, stat 2 only at q=STRIP_P-1).  Staged host-side because
    # engines can't address partition starts off the 0/32/64/96 grid.
    consts = nc.dram_tensor("consts", [128, N_SLICES + 3], bf16,
                            kind="ExternalInput").ap()
    summ = nc.dram_tensor("summ", [3, N_SLICES], f32,
                          kind="ExternalOutput").ap()

    with tile.TileContext(nc) as tc:
        with (
            tc.tile_pool(name="singles", bufs=1) as singles,
            tc.tile_pool(name="loads", bufs=8) as loads,
            tc.tile_pool(name="relus", bufs=8) as relus,
            tc.tile_pool(name="rms", bufs=8) as rms,
            tc.tile_pool(name="outs", bufs=8) as outs,
            tc.tile_pool(name="psum", bufs=4, space="PSUM") as psum,
        ):
            bias_t = singles.tile([128, 1], f32)
            # slices 0..3 are pred rows (T=0.3), 4..7 exp rows (T=0.5)
            nc.vector.memset(bias_t[0:64], -THRESHOLDS[0])
            nc.vector.memset(bias_t[64:128], -THRESHOLDS[1])
            consts_t = singles.tile([128, N_SLICES + 3], bf16)
            nc.sync.dma_start(out=consts_t, in_=consts)
            sel = consts_t[:, 0:N_SLICES]
            mask_t = consts_t[:, N_SLICES:N_SLICES + 3]

            def body(_iv=None):
                for _u in range(unroll):
                    ld = loads.tile([128, STRIP_F], f32)
                    nc.sync.dma_start(out=ld, in_=strips)
                    rl = relus.tile([128, STRIP_F], bf16)
                    nc.scalar.activation(
                        out=rl, in_=ld,
                        func=mybir.ActivationFunctionType.Relu,
                        bias=bias_t, scale=1.0)
                    rm = rms.tile([128, 3], bf16)
                    nc.vector.reduce_max(rm[:, 0:1], rl,
                                         axis=mybir.AxisListType.X)
                    # the two corner 16-channel blocks as one strided view
                    rl3 = rl.rearrange("p (j f) -> p j f", f=C)
                    n_j = STRIP_F // C  # 32 chunks; chunks 0 and 31 = corners
                    nc.vector.reduce_max(rm[:, 1:3], rl3[:, ::n_j - 1, :],
                                         axis=mybir.AxisListType.X)
                    rmm = rms.tile([128, 3], bf16, tag="rmm")
                    nc.vector.tensor_tensor(out=rmm, in0=rm, in1=mask_t,
                                            op=mybir.AluOpType.mult)
                    ps = psum.tile([3, N_SLICES], f32)
                    nc.tensor.matmul(ps, rmm, sel, start=True, stop=True)
                    sm = outs.tile([3, N_SLICES], f32)
                    nc.scalar.activation(
                        out=sm, in_=ps,
                        func=mybir.ActivationFunctionType.Copy)
                    nc.scalar.dma_start(out=summ, in_=sm)

            if repeat == 1:
                body()
            else:
                with tc.For_i(0, repeat, 1,
                              hint_engines=(mybir.EngineType.PE,)) as iv:
                    body(iv)

    nc.compile()
    return nc


def _strip_consts():
    import ml_dtypes
    sel = np.zeros((128, N_SLICES), dtype=np.float32)
    mask = np.zeros((128, 3), dtype=np.float32)
    mask[:, 0] = 1.0
    for g in range(N_SLICES):
        sel[g * STRIP_P:(g + 1) * STRIP_P, g] = 1.0
        mask[g * STRIP_P, 1] = 1.0
        mask[(g + 1) * STRIP_P - 1, 2] = 1.0
    return np.ascontiguousarray(
        np.concatenate([sel, mask], axis=1)).astype(ml_dtypes.bfloat16)


def _shard_strips(p, e):
    """Host gather of the edge rows: [N_CORES] maps of strips [128, 512]."""
    rows = np.asarray(STRIP_ROWS)
    p3 = p.reshape(B, H, FREE)
    e3 = e.reshape(B, H, FREE)
    consts = _strip_consts()
    maps = []
    for c in range(N_CORES):
        blocks = [arr[c * SAMPLES_PER_CORE + s][rows]
                  for arr in (p3, e3) for s in range(SAMPLES_PER_CORE)]
        strips = np.ascontiguousarray(
            np.concatenate(blocks, axis=0)).reshape(128, STRIP_F)
        maps.append({"strips": strips, "consts": consts})
    return maps


def _combine_strip(results):
    """Host epilogue for phase 1. Returns the scalar result, or None if any
    bbox corner is not provable from the edge strips (-> full fallback).
    summ [3, g]: g = st*2 + j; stat 0 = row evidence, 1 = col 0, 2 = col 511.
    Entries are sums of non-negative relu(v - T) evidence: > 0 iff some
    pixel of that row (resp. corner column block) exceeds the threshold."""
    f = np.float32
    penalties = []
    for core in range(N_CORES):
        sm = results[core]["summ"]  # [3, N_SLICES]
        boxes = []
        for st in range(N_ST):
            g0, g1 = st * N_STRIP_ROWS, st * N_STRIP_ROWS + 1
            if not sm[0, g0] > 0:
                return None  # row 0 empty -> y_min unknown from strips
            if not sm[0, g1] > 0:
                return None  # row 511 empty -> y_max unknown
            if not (sm[1, g0] > 0 or sm[1, g1] > 0):
                return None  # column 0 not proven nonempty
            if not (sm[2, g0] > 0 or sm[2, g1] > 0):
                return None  # column 511 not proven nonempty
            boxes.append((STRIP_ROWS[0], 0, STRIP_ROWS[1], W - 1))
        for s in range(SAMPLES_PER_CORE):
            py1, px1, py2, px2 = boxes[s]                      # pred
            ty1, tx1, ty2, tx2 = boxes[SAMPLES_PER_CORE + s]   # exp
            pred_area = f((py2 - py1 + 1) * (px2 - px1 + 1))
            true_area = f((ty2 - ty1 + 1) * (tx2 - tx1 + 1))
            area_penalty = f(max(f(0.0), f(pred_area - true_area))) / f(true_area + f(1.0))
            pcy, pcx = f(py1 + py2) / f(2.0), f(px1 + px2) / f(2.0)
            tcy, tcx = f(ty1 + ty2) / f(2.0), f(tx1 + tx2) / f(2.0)
            center_offset = np.sqrt(np.square(f(pcy - tcy)) + np.square(f(pcx - tcx))) / f(20.0)
            penalties.append(f(area_penalty + center_offset))
    mean = np.mean(np.asarray(penalties, dtype=np.float32), dtype=np.float32)
    return np.asarray(np.float32(PENALTY_WEIGHT) * mean, dtype=np.float32)


def _shard_inputs(prediction_probs, expected_onehot):
    p = np.ascontiguousarray(np.asarray(prediction_probs), dtype=np.float32)
    e = np.ascontiguousarray(np.asarray(expected_onehot), dtype=np.float32)
    p = p.reshape(N_CORES, SAMPLES_PER_CORE * TILES_PER_SAMPLE, 128, FREE)
    e = e.reshape(N_CORES, SAMPLES_PER_CORE * TILES_PER_SAMPLE, 128, FREE)
    return [{"pred": p[c], "exp": e[c]} for c in range(N_CORES)]


def _bbox_from_any(row_any, col_any):
    ys = np.nonzero(row_any)[0]
    xs = np.nonzero(col_any)[0]
    if ys.size == 0:
        return 0, 0, 1, 1
    return int(ys[0]), int(xs[0]), int(ys[-1]), int(xs[-1])


def _combine(results):
    """Host epilogue: exact bbox/penalty math from row/col summaries."""
    f = np.float32
    penalties = []
    for core in range(N_CORES):
        rows = results[core]["rows"]  # [128, 16]
        cols = results[core]["cols"]  # [4, 512]
        for s in range(SAMPLES_PER_CORE):
            boxes = []
            for tensor_idx in range(2):
                k0 = tensor_idx * 8 + s * 4
                row_any = rows[:, k0 : k0 + 4].T.ravel() > 0  # y = t*128 + p
                col_any = cols[tensor_idx * SAMPLES_PER_CORE + s] > 0
                boxes.append(_bbox_from_any(row_any, col_any))
            (py1, px1, py2, px2), (ty1, tx1, ty2, tx2) = boxes
            pred_area = f((py2 - py1 + 1) * (px2 - px1 + 1))
            true_area = f((ty2 - ty1 + 1) * (tx2 - tx1 + 1))
            area_penalty = f(max(f(0.0), f(pred_area - true_area))) / f(true_area + f(1.0))
            pcy, pcx = f(py1 + py2) / f(2.0), f(px1 + px2) / f(2.0)
            tcy, tcx = f(ty1 + ty2) / f(2.0), f(tx1 + tx2) / f(2.0)
            center_offset = np.sqrt(np.square(f(pcy - tcy)) + np.square(f(pcx - tcx))) / f(20.0)
            penalties.append(f(area_penalty + center_offset))
    mean = np.mean(np.asarray(penalties, dtype=np.float32), dtype=np.float32)
    return np.asarray(np.float32(PENALTY_WEIGHT) * mean, dtype=np.float32)


_NC_CACHE = {}


def _run_spmd(nc, in_maps):
    last_exc = None
    for attempt in range(3):  # the axon device occasionally flakes transiently
        try:
            return run_bass_kernel_spmd(nc, in_maps, core_ids=list(range(N_CORES)))
        except Exception as e:  # noqa: BLE001
            last_exc = e
            try:
                # an NRT_EXEC_UNIT_UNRECOVERABLE poisons the PJRT mesh for
                # the whole process; dropping the backend forces a reconnect
                import jax.extend.backend

                jax.extend.backend.clear_backends()
            except Exception:  # noqa: BLE001
                pass
            time.sleep(5.0)
    raise last_exc


def kernel(prediction_probs, expected_onehot):
    p = np.ascontiguousarray(np.asarray(prediction_probs), dtype=np.float32)
    e = np.ascontiguousarray(np.asarray(expected_onehot), dtype=np.float32)

    # Phase 1: edge-strip kernel (reads 4 MiB total instead of 512 MiB).
    if "strip" not in _NC_CACHE:
        _NC_CACHE["strip"] = build_strip_nc()
    res = _run_spmd(_NC_CACHE["strip"], _shard_strips(p, e))
    out = _combine_strip(res.results)
    if out is not None:
        return out

    # Phase 2 (exact fallback for unresolved corners): full-read kernel.
    if "nc" not in _NC_CACHE:
        _NC_CACHE["nc"] = build_nc()
    res = _run_spmd(_NC_CACHE["nc"], _shard_inputs(p, e))
    return _combine(res.results)

